# revision 6
# baseline (speedup 1.0000x reference)
"""Trainium2 Bass kernel for nn_MultiLayer_BTreeLSTM (2-layer bidirectional
tree-LSTM over a complete binary heap of N=16384 nodes, F=512, H=256).

Strategy: the heap tree is split into 8 subtrees rooted at level 3 (nodes
7..14), one per NeuronCore.  Each core holds its subtree's nodes (levels
3..13, 2047 nodes) plus a replicated copy of the 7 top nodes and the single
level-14 node (16383).  All recurrences are level-batched: within a level the
child/parent gathers are stride-2 / repeat-2 access patterns in a level-major
layout, so every step is a dense GEMM + gate math.  The up-sweep needs one
8-core AllGather (the 8 subtree-root states) per layer; the down-sweep needs
no communication (the 7-node top is computed redundantly on every core).

Data layout on device is feature-major: [feature partition, node column].
Per-core column map (R = 2055 columns):
  col 0..6   : global nodes 0..6 (top, replicated)
  col 7      : global node 16383 (the only level-14 node; masked on cores>0)
  col 8 + t  : subtree nodes, level-major; level d (3..13) occupies columns
               [8 + 2^(d-3) - 1, 8 + 2^(d-2) - 1), i.e. m_d = 2^(d-3) columns.
Matmuls run in bf16 with fp32 PSUM accumulation; cell state c and all gate
math stay fp32.
"""

import numpy as np
import ml_dtypes

N = 16384
F = 512
H = 256
NLAYER = 2
NCORE = 8
R = 2055
BF16 = ml_dtypes.bfloat16


def _sub_off(d):
    """Column offset of subtree level d (3 <= d <= 13)."""
    return 8 + (1 << (d - 3)) - 1


def _sub_m(d):
    return 1 << (d - 3)


def _col_nodes_for_core(k):
    """Global node index for each per-core column."""
    cols = np.empty(R, np.int64)
    cols[0:7] = np.arange(7)
    cols[7] = 16383
    p = 8
    for d in range(3, 14):
        m = _sub_m(d)
        start = (1 << d) - 1 + k * m
        cols[p:p + m] = np.arange(start, start + m)
        p += m
    assert p == R
    return cols


def build_nc():
    """Build the 8-core SPMD Bass/Tile program.  Returns the Bacc object."""
    from contextlib import ExitStack
    import concourse.bacc as bacc
    import concourse.mybir as mybir
    import concourse.tile as tile
    from concourse.bass import ts

    f32 = mybir.dt.float32
    bf16 = mybir.dt.bfloat16
    AF = mybir.ActivationFunctionType

    nc = bacc.Bacc("TRN2", num_devices=NCORE, debug=False)

    # ---------------- DRAM I/O ----------------
    feats_in = nc.dram_tensor("feats_in", [F, R], bf16, kind="ExternalInput").ap()
    wdr = {}
    for l in range(NLAYER):
        wdr[f"wup{l}"] = nc.dram_tensor(f"wup{l}", [1024, 1536], bf16, kind="ExternalInput").ap()
        wdr[f"wpf{l}"] = nc.dram_tensor(f"wpf{l}", [512, 256], bf16, kind="ExternalInput").ap()
        wdr[f"wxb{l}"] = nc.dram_tensor(f"wxb{l}", [512, 1280], bf16, kind="ExternalInput").ap()
        wdr[f"whb{l}"] = nc.dram_tensor(f"whb{l}", [256, 1280], bf16, kind="ExternalInput").ap()
        wdr[f"wpb{l}"] = nc.dram_tensor(f"wpb{l}", [512, 256], bf16, kind="ExternalInput").ap()
    biases_d = nc.dram_tensor("biases", [128, 52], f32, kind="ExternalInput").ap()
    coreconst_d = nc.dram_tensor("coreconst", [128, 8], f32, kind="ExternalInput").ap()
    out_d = nc.dram_tensor("out", [F, R], f32, kind="ExternalOutput").ap()

    with tile.TileContext(nc) as tc, ExitStack() as ctx:
        pool_xa = ctx.enter_context(tc.tile_pool(name="xa", bufs=1))
        pool_xb = ctx.enter_context(tc.tile_pool(name="xb", bufs=1))
        pool_c = ctx.enter_context(tc.tile_pool(name="cst", bufs=1))
        pool_w = ctx.enter_context(tc.tile_pool(name="wts", bufs=1))
        pool_misc = ctx.enter_context(tc.tile_pool(name="misc", bufs=1))
        pool_g = ctx.enter_context(tc.tile_pool(name="gat", bufs=2))
        pool_px = ctx.enter_context(tc.tile_pool(name="pxp", bufs=2))
        pool_tmp = ctx.enter_context(tc.tile_pool(name="tmp", bufs=2))
        pool_st = ctx.enter_context(tc.tile_pool(name="stg", bufs=3))
        pool_ps = ctx.enter_context(tc.tile_pool(name="psg", bufs=4, space="PSUM"))
        pool_psx = ctx.enter_context(tc.tile_pool(name="psx", bufs=2, space="PSUM"))

        # ---------------- persistent SBUF ----------------
        featsA = [pool_xa.tile([128, R], bf16, tag=f"xa{t}", name=f"fa{t}") for t in range(4)]
        featsB = [pool_xb.tile([128, R], bf16, tag=f"xb{t}", name=f"fb{t}") for t in range(4)]
        cu = [pool_c.tile([128, R], f32, tag=f"cu{t}", name=f"cu{t}") for t in range(2)]
        cd = [pool_c.tile([128, R], f32, tag=f"cd{t}", name=f"cd{t}") for t in range(2)]
        biases = pool_misc.tile([128, 52], f32, name="biases_sb")
        coreconst = pool_misc.tile([128, 8], f32, name="coreconst_sb")

        for t in range(4):
            nc.sync.dma_start(featsA[t][:], feats_in[ts(t, 128), :])
        nc.sync.dma_start(biases[:], biases_d[:])
        nc.sync.dma_start(coreconst[:], coreconst_d[:])
        maskv = coreconst[:, 0:1]
        sel4 = coreconst[:, 1:5]

        # ---------------- level-step emitters ----------------
        def up_level(wup, wpf, bcol, src, hbuf, cbuf, off, m, children,
                     extra_fix=None, store_out=False, root32=None):
            """One up-sweep level: gates for columns [off, off+m).

            children: None (leaf) or (chh[2] bf16 tiles, chc[2] f32 tiles, choff)
              with child columns [choff, choff+2m) (left=even offset).
            extra_fix: (hfe_m[2] bf16, ce_m[2] f32) single-column fix at col off.
            root32: optional [2] f32 [128,1] tiles to also receive hf (fp32).
            """
            for n0 in range(0, m, 512):
                ncv = min(512, m - n0)
                noff = off + n0
                gt = []
                for i in range(12):
                    ps = pool_ps.tile([128, ncv], f32, tag="gps", name="gps")
                    mms = []
                    for t in range(4):
                        mms.append((wup[t][:, ts(i, 128)],
                                    src[t][:, noff:noff + ncv]))
                    if children is not None:
                        chh, chc, choff = children
                        base = choff + 2 * n0
                        for t in range(4):
                            tile_idx, parity = t % 2, t // 2
                            s0 = base + parity
                            rhs = chh[tile_idx][:, s0: s0 + 2 * ncv - 1: 2]
                            mms.append((wup[4 + t][:, ts(i, 128)], rhs))
                    if extra_fix is not None and n0 == 0:
                        hfe_m, _ = extra_fix
                        for t in range(2):
                            mms.append((wup[4 + t][:, ts(i, 128)], hfe_m[t][:],
                                        ps[:, 0:1]))
                    for j, mm in enumerate(mms):
                        dst = mm[2] if len(mm) > 2 else ps[:]
                        nc.tensor.matmul(dst, mm[0], mm[1],
                                         start=(j == 0), stop=(j == len(mms) - 1))
                    g = pool_g.tile([128, ncv], f32, tag=f"g{i}", name=f"g{i}")
                    func = AF.Tanh if i in (8, 9) else AF.Sigmoid
                    nc.scalar.activation(g[:], ps[:], func,
                                         bias=biases[:, bcol + i:bcol + i + 1])
                    gt.append(g)
                pxt = []
                for t2 in range(2):
                    psx = pool_psx.tile([128, ncv], f32, tag="pxps", name="pxps")
                    for t in range(4):
                        nc.tensor.matmul(psx[:], wpf[t][:, ts(t2, 128)],
                                         src[t][:, noff:noff + ncv],
                                         start=(t == 0), stop=(t == 3))
                    px = pool_px.tile([128, ncv], f32, tag=f"px{t2}", name=f"px{t2}")
                    nc.scalar.activation(px[:], psx[:], AF.Identity,
                                         bias=biases[:, bcol + 12 + t2:bcol + 13 + t2])
                    pxt.append(px)
                for t2 in range(2):
                    ig, og, flg, frg, ug, rg = (gt[0 + t2], gt[2 + t2], gt[4 + t2],
                                                gt[6 + t2], gt[8 + t2], gt[10 + t2])
                    cdst = cbuf[t2][:, noff:noff + ncv]
                    nc.vector.tensor_mul(cdst, ig[:], ug[:])
                    if children is not None:
                        chh, chc, choff = children
                        base = choff + 2 * n0
                        for parity, fgate in ((0, flg), (1, frg)):
                            s0 = base + parity
                            cch = chc[t2][:, s0: s0 + 2 * ncv - 1: 2]
                            tmp = pool_tmp.tile([128, ncv], f32, tag=f"tmpa{t2}", name="tmpa")
                            nc.vector.tensor_mul(tmp[:], fgate[:], cch)
                            nc.vector.tensor_add(cdst, cdst, tmp[:])
                    if extra_fix is not None and n0 == 0:
                        _, ce_m = extra_fix
                        tmp1 = pool_tmp.tile([128, 1], f32, tag=f"tmpe{t2}", name="tmpe")
                        nc.vector.tensor_mul(tmp1[:], flg[:, 0:1], ce_m[t2][:])
                        nc.vector.tensor_add(cbuf[t2][:, noff:noff + 1],
                                             cbuf[t2][:, noff:noff + 1], tmp1[:])
                    th = pool_tmp.tile([128, ncv], f32, tag=f"th{t2}", name="th")
                    nc.scalar.activation(th[:], cdst, AF.Tanh)
                    hh = pool_tmp.tile([128, ncv], f32, tag=f"hh{t2}", name="hh")
                    nc.vector.tensor_mul(hh[:], og[:], th[:])
                    nc.vector.tensor_sub(hh[:], hh[:], pxt[t2][:])
                    nc.vector.tensor_mul(hh[:], rg[:], hh[:])
                    # hf = px + r*(h - px)
                    if store_out:
                        st = pool_st.tile([128, ncv], f32, tag=f"st{t2}", name="st")
                        nc.vector.tensor_add(st[:], pxt[t2][:], hh[:])
                        nc.sync.dma_start(out_d[ts(t2, 128), noff:noff + ncv], st[:])
                        nc.vector.tensor_copy(hbuf[t2][:, noff:noff + ncv], st[:])
                        if root32 is not None:
                            nc.vector.tensor_copy(root32[t2][:], st[:, 0:1])
                    else:
                        nc.vector.tensor_add(hbuf[t2][:, noff:noff + ncv],
                                             pxt[t2][:], hh[:])
                        if root32 is not None:
                            nc.vector.tensor_add(root32[t2][:], pxt[t2][:, 0:1],
                                                 hh[:, 0:1])

        def down_level(wxb, whb, wpb, bcol, src, hbuf, cbuf, off, m, parents,
                       store_out=False):
            """One down-sweep level: gates for columns [off, off+m).

            parents: None (root) or (ph[2] bf16, pc[2] f32, poff); parent of
            col off+j is column poff + j//2 (or poff when m == 1).
            """
            for n0 in range(0, m, 512):
                ncv = min(512, m - n0)
                noff = off + n0
                np2 = max(1, ncv // 2)
                gt = []
                for i in range(10):
                    ps = pool_ps.tile([128, ncv], f32, tag="gps", name="gpsd")
                    mms = [(wxb[t][:, ts(i, 128)], src[t][:, noff:noff + ncv])
                           for t in range(4)]
                    if parents is not None:
                        ph, pc, poff = parents
                        for t in range(2):
                            if m == 1:
                                rhs = ph[t][:, poff:poff + 1]
                            else:
                                p0 = poff + n0 // 2
                                rhs = ph[t][:, p0:p0 + np2].broadcast_to([128, np2, 2])
                            mms.append((whb[t][:, ts(i, 128)], rhs))
                    for j, mm in enumerate(mms):
                        nc.tensor.matmul(ps[:], mm[0], mm[1],
                                         start=(j == 0), stop=(j == len(mms) - 1))
                    g = pool_g.tile([128, ncv], f32, tag=f"g{i}", name=f"gd{i}")
                    func = AF.Tanh if i in (6, 7) else AF.Sigmoid
                    nc.scalar.activation(g[:], ps[:], func,
                                         bias=biases[:, bcol + i:bcol + i + 1])
                    gt.append(g)
                pxt = []
                for t2 in range(2):
                    psx = pool_psx.tile([128, ncv], f32, tag="pxps", name="pxpsd")
                    for t in range(4):
                        nc.tensor.matmul(psx[:], wpb[t][:, ts(t2, 128)],
                                         src[t][:, noff:noff + ncv],
                                         start=(t == 0), stop=(t == 3))
                    px = pool_px.tile([128, ncv], f32, tag=f"px{t2}", name=f"pxd{t2}")
                    nc.scalar.activation(px[:], psx[:], AF.Identity,
                                         bias=biases[:, bcol + 10 + t2:bcol + 11 + t2])
                    pxt.append(px)
                for t2 in range(2):
                    ig, og, fg, ug, rg = (gt[0 + t2], gt[2 + t2], gt[4 + t2],
                                          gt[6 + t2], gt[8 + t2])
                    cdst = cbuf[t2][:, noff:noff + ncv]
                    nc.vector.tensor_mul(cdst, ig[:], ug[:])
                    if parents is not None:
                        ph, pc, poff = parents
                        tmp = pool_tmp.tile([128, ncv], f32, tag=f"tmpa{t2}", name="tmpad")
                        if m == 1:
                            cpar = pc[t2][:, poff:poff + 1]
                            nc.vector.tensor_mul(tmp[:], fg[:], cpar)
                            nc.vector.tensor_add(cdst, cdst, tmp[:])
                        else:
                            p0 = poff + n0 // 2
                            cpar = pc[t2][:, p0:p0 + np2].broadcast_to([128, np2, 2])
                            fg3 = fg[:].rearrange("p (a b) -> p a b", b=2)
                            tmp3 = tmp[:].rearrange("p (a b) -> p a b", b=2)
                            nc.vector.tensor_mul(tmp3, fg3, cpar)
                            nc.vector.tensor_add(cdst, cdst, tmp[:])
                    th = pool_tmp.tile([128, ncv], f32, tag=f"th{t2}", name="thd")
                    nc.scalar.activation(th[:], cdst, AF.Tanh)
                    hh = pool_tmp.tile([128, ncv], f32, tag=f"hh{t2}", name="hhd")
                    nc.vector.tensor_mul(hh[:], og[:], th[:])
                    nc.vector.tensor_sub(hh[:], hh[:], pxt[t2][:])
                    nc.vector.tensor_mul(hh[:], rg[:], hh[:])
                    if store_out:
                        st = pool_st.tile([128, ncv], f32, tag=f"st{t2}", name="std")
                        nc.vector.tensor_add(st[:], pxt[t2][:], hh[:])
                        nc.sync.dma_start(out_d[ts(2 + t2, 128), noff:noff + ncv], st[:])
                        nc.vector.tensor_copy(hbuf[t2][:, noff:noff + ncv], st[:])
                    else:
                        nc.vector.tensor_add(hbuf[t2][:, noff:noff + ncv],
                                             pxt[t2][:], hh[:])

        # ---------------- the two layers ----------------
        h2u = h2d = None
        for l in range(NLAYER):
            wup = [pool_w.tile([128, 1536], bf16, tag=f"wup{t}", name=f"wup{t}") for t in range(8)]
            wpf = [pool_w.tile([128, 256], bf16, tag=f"wpf{t}", name=f"wpf{t}") for t in range(4)]
            wxb = [pool_w.tile([128, 1280], bf16, tag=f"wxb{t}", name=f"wxb{t}") for t in range(4)]
            whb = [pool_w.tile([128, 1280], bf16, tag=f"whb{t}", name=f"whb{t}") for t in range(2)]
            wpb = [pool_w.tile([128, 256], bf16, tag=f"wpb{t}", name=f"wpb{t}") for t in range(4)]
            for t in range(8):
                nc.sync.dma_start(wup[t][:], wdr[f"wup{l}"][ts(t, 128), :])
            for t in range(4):
                nc.sync.dma_start(wpf[t][:], wdr[f"wpf{l}"][ts(t, 128), :])
                nc.sync.dma_start(wxb[t][:], wdr[f"wxb{l}"][ts(t, 128), :])
                nc.sync.dma_start(wpb[t][:], wdr[f"wpb{l}"][ts(t, 128), :])
            for t in range(2):
                nc.sync.dma_start(whb[t][:], wdr[f"whb{l}"][ts(t, 128), :])

            if l == 0:
                src = featsA
                hu, hd = featsB[0:2], featsB[2:4]
            else:
                h2u = [pool_xa.tile([128, R], bf16, tag=f"xa{t}", name=f"h2u{t}") for t in range(2)]
                h2d = [pool_xa.tile([128, R], bf16, tag=f"xa{2 + t}", name=f"h2d{t}") for t in range(2)]
                src = featsB
                hu, hd = h2u, h2d
            so = (l == NLAYER - 1)
            bcu = l * 26
            bcd = l * 26 + 14

            # ---- UP sweep ----
            hfe32 = [pool_misc.tile([128, 1], f32, tag=f"hfe32{t}", name="hfe32") for t in range(2)]
            up_level(wup, wpf, bcu, src, hu, cu, 7, 1, None,
                     store_out=so, root32=hfe32)
            hfe_m = [pool_misc.tile([128, 1], bf16, tag=f"hfem{t}", name="hfem") for t in range(2)]
            ce_m = [pool_misc.tile([128, 1], f32, tag=f"cem{t}", name="cem") for t in range(2)]
            for t in range(2):
                nc.scalar.mul(hfe_m[t][:], hfe32[t][:], maskv)
                nc.scalar.mul(ce_m[t][:], cu[t][:, 7:8], maskv)
            up_level(wup, wpf, bcu, src, hu, cu, _sub_off(13), 1024, None,
                     extra_fix=(hfe_m, ce_m), store_out=so)
            root_hf32 = [pool_misc.tile([128, 1], f32, tag=f"rhf32{t}", name="roothf") for t in range(2)]
            for d in range(12, 2, -1):
                up_level(wup, wpf, bcu, src, hu, cu, _sub_off(d), _sub_m(d),
                         (hu, cu, _sub_off(d + 1)), store_out=so,
                         root32=(root_hf32 if d == 3 else None))

            # ---- AllGather of the 8 subtree-root (c, hf) states ----
            cc_in, _f1 = tc.tile([1, 512], f32, space="DRAM", name=f"cc_in{l}")
            ctx.callback(_f1)
            cc_out, _f2 = tc.tile([8, 512], f32, space="DRAM", addr_space="Shared",
                             name=f"cc_out{l}")
            ctx.callback(_f2)
            for t in range(2):
                nc.sync.dma_start(cc_in[0:1, ts(t, 128)].rearrange("o p -> p o"),
                                  cu[t][:, 8:9])
                nc.sync.dma_start(cc_in[0:1, ts(2 + t, 128)].rearrange("o p -> p o"),
                                  root_hf32[t][:])
            import concourse.mybir as _mybir
            nc.gpsimd.collective_compute(
                "AllGather", _mybir.AluOpType.bypass,
                replica_groups=[list(range(NCORE))],
                ins=[cc_in[:]], outs=[cc_out[:]])
            rc = [pool_misc.tile([128, 8], f32, tag=f"rc{t}", name="rc") for t in range(2)]
            rhf = [pool_misc.tile([128, 8], f32, tag=f"rhf{t}", name="rhf") for t in range(2)]
            rhb = [pool_misc.tile([128, 8], bf16, tag=f"rhb{t}", name="rhb") for t in range(2)]
            for t in range(2):
                nc.sync.dma_start(rc[t][:], cc_out[:, ts(t, 128)].rearrange("j p -> p j"))
                nc.sync.dma_start(rhf[t][:], cc_out[:, ts(2 + t, 128)].rearrange("j p -> p j"))
                nc.vector.tensor_copy(rhb[t][:], rhf[t][:])

            up_level(wup, wpf, bcu, src, hu, cu, 3, 4, (rhb, rc, 0), store_out=so)
            up_level(wup, wpf, bcu, src, hu, cu, 1, 2, (hu, cu, 3), store_out=so)
            up_level(wup, wpf, bcu, src, hu, cu, 0, 1, (hu, cu, 1), store_out=so)

            # ---- DOWN sweep ----
            down_level(wxb, whb, wpb, bcd, src, hd, cd, 0, 1, None, store_out=so)
            down_level(wxb, whb, wpb, bcd, src, hd, cd, 1, 2, (hd, cd, 0), store_out=so)
            down_level(wxb, whb, wpb, bcd, src, hd, cd, 3, 4, (hd, cd, 1), store_out=so)
            # subtree root (col 8): parent = top level-2 column 3 + k//2 via one-hot
            hpar = [pool_misc.tile([128, 1], f32, tag=f"hpar{t}", name="hpar") for t in range(2)]
            hparb = [pool_misc.tile([128, 1], bf16, tag=f"hparb{t}", name="hparb") for t in range(2)]
            cpar = [pool_misc.tile([128, 1], f32, tag=f"cpar{t}", name="cpar") for t in range(2)]
            for t in range(2):
                tsel = pool_tmp.tile([128, 4], f32, tag=f"tsel{t}", name="tsel")
                nc.vector.tensor_copy(tsel[:], hd[t][:, 3:7])
                nc.vector.tensor_mul(tsel[:], tsel[:], sel4)
                nc.vector.reduce_sum(hpar[t][:], tsel[:], axis=_mybir.AxisListType.X)
                nc.vector.tensor_copy(hparb[t][:], hpar[t][:])
                tsel2 = pool_tmp.tile([128, 4], f32, tag=f"tsel2{t}", name="tsel2")
                nc.vector.tensor_mul(tsel2[:], cd[t][:, 3:7], sel4)
                nc.vector.reduce_sum(cpar[t][:], tsel2[:], axis=_mybir.AxisListType.X)
            down_level(wxb, whb, wpb, bcd, src, hd, cd, 8, 1, (hparb, cpar, 0),
                       store_out=so)
            for d in range(4, 14):
                down_level(wxb, whb, wpb, bcd, src, hd, cd, _sub_off(d), _sub_m(d),
                           (hd, cd, _sub_off(d - 1)), store_out=so)
            # level-14 node (col 7): parent is subtree level-13 col 0 (col 1031)
            down_level(wxb, whb, wpb, bcd, src, hd, cd, 7, 1,
                       (hd, cd, _sub_off(13)), store_out=so)

    nc.compile()
    return nc


def _prep_inputs(features, Wxf, bxf, Wlf, blf, Wrf, brf, Wpf, bpf,
                 Wxb, bxb, Whb, bhb, Wpb, bpb):
    """Host-side sharding: build the per-core input maps."""
    features = np.asarray(features, np.float32)
    in_maps = []
    shared = {}
    for l in range(NLAYER):
        wcomb = np.concatenate([np.asarray(Wlf[l]), np.asarray(Wrf[l])], axis=1)
        wup = np.concatenate([np.asarray(Wxf[l]).T, wcomb.T], axis=0)
        shared[f"wup{l}"] = np.ascontiguousarray(wup).astype(BF16)
        shared[f"wpf{l}"] = np.ascontiguousarray(np.asarray(Wpf[l]).T).astype(BF16)
        shared[f"wxb{l}"] = np.ascontiguousarray(np.asarray(Wxb[l]).T).astype(BF16)
        shared[f"whb{l}"] = np.ascontiguousarray(np.asarray(Whb[l]).T).astype(BF16)
        shared[f"wpb{l}"] = np.ascontiguousarray(np.asarray(Wpb[l]).T).astype(BF16)
    biases = np.zeros((128, 52), np.float32)
    for l in range(NLAYER):
        bup = (np.asarray(bxf[l]) + np.asarray(blf[l]) + np.asarray(brf[l])).astype(np.float32)
        bdn = (np.asarray(bxb[l]) + np.asarray(bhb[l])).astype(np.float32)
        biases[:, l * 26 + 0:l * 26 + 12] = bup.reshape(12, 128).T
        biases[:, l * 26 + 12:l * 26 + 14] = np.asarray(bpf[l], np.float32).reshape(2, 128).T
        biases[:, l * 26 + 14:l * 26 + 24] = bdn.reshape(10, 128).T
        biases[:, l * 26 + 24:l * 26 + 26] = np.asarray(bpb[l], np.float32).reshape(2, 128).T
    for k in range(NCORE):
        cols = _col_nodes_for_core(k)
        fk = np.ascontiguousarray(features[cols, :].T).astype(BF16)
        cc = np.zeros((128, 8), np.float32)
        cc[:, 0] = 1.0 if k == 0 else 0.0
        cc[:, 1 + (k // 2)] = 1.0
        m = dict(shared)
        m["feats_in"] = fk
        m["biases"] = biases
        m["coreconst"] = cc
        in_maps.append(m)
    return in_maps


def _assemble_output(results):
    """Gather per-core [F, R] outputs back to the full [N, F] array."""
    out = np.empty((N, F), np.float32)
    for k in range(NCORE):
        cols = _col_nodes_for_core(k)
        ok = results[k]["out"]  # [F, R]
        if k == 0:
            out[cols, :] = ok.T
        else:
            out[cols[8:], :] = ok.T[8:, :]
    return out


def kernel(features, left, right, parent, Wxf, bxf, Wlf, blf, Wrf, brf,
           Wpf, bpf, Wxb, bxb, Whb, bhb, Wpb, bpb):
    from concourse import bass_utils
    nc = build_nc()
    in_maps = _prep_inputs(features, Wxf, bxf, Wlf, blf, Wrf, brf, Wpf, bpf,
                           Wxb, bxb, Whb, bhb, Wpb, bpb)
    res = bass_utils.run_bass_kernel_spmd(nc, in_maps, core_ids=list(range(NCORE)))
    return _assemble_output(res.results)


# revision 7
# speedup vs baseline: 1.2330x; 1.2330x over previous
"""Trainium2 Bass kernel for nn_MultiLayer_BTreeLSTM (2-layer bidirectional
tree-LSTM over a complete binary heap of N=16384 nodes, F=512, H=256).

Strategy: the heap tree is split into 8 subtrees rooted at level 3 (nodes
7..14), one per NeuronCore.  Each core holds its subtree's nodes (levels
3..13, 2047 nodes) plus a replicated copy of the 7 top nodes and the single
level-14 node (16383).  All recurrences are level-batched: within a level the
child/parent gathers are stride-2 / repeat-2 access patterns in a level-major
layout, so every step is a dense GEMM + gate math.  The up-sweep needs one
8-core AllGather (the 8 subtree-root states) per layer; the down-sweep needs
no communication (the 7-node top is computed redundantly on every core).

Data layout on device is feature-major: [feature partition, node column].
Per-core column map (R = 2055 columns):
  col 0..6   : global nodes 0..6 (top, replicated)
  col 7      : global node 16383 (the only level-14 node; masked on cores>0)
  col 8 + t  : subtree nodes, level-major; level d (3..13) occupies columns
               [8 + 2^(d-3) - 1, 8 + 2^(d-2) - 1), i.e. m_d = 2^(d-3) columns.
Matmuls run in bf16 with fp32 PSUM accumulation; cell state c and all gate
math stay fp32.
"""

import numpy as np
import ml_dtypes

N = 16384
F = 512
H = 256
NLAYER = 2
NCORE = 8
R = 2055
BF16 = ml_dtypes.bfloat16


def _sub_off(d):
    """Column offset of subtree level d (3 <= d <= 13)."""
    return 8 + (1 << (d - 3)) - 1


def _sub_m(d):
    return 1 << (d - 3)


def _col_nodes_for_core(k):
    """Global node index for each per-core column."""
    cols = np.empty(R, np.int64)
    cols[0:7] = np.arange(7)
    cols[7] = 16383
    p = 8
    for d in range(3, 14):
        m = _sub_m(d)
        start = (1 << d) - 1 + k * m
        cols[p:p + m] = np.arange(start, start + m)
        p += m
    assert p == R
    return cols


def build_nc():
    """Build the 8-core SPMD Bass/Tile program.  Returns the Bacc object."""
    from contextlib import ExitStack
    import concourse.bacc as bacc
    import concourse.mybir as mybir
    import concourse.tile as tile
    from concourse.bass import ts

    f32 = mybir.dt.float32
    bf16 = mybir.dt.bfloat16
    AF = mybir.ActivationFunctionType

    nc = bacc.Bacc("TRN2", num_devices=NCORE, debug=False)

    # ---------------- DRAM I/O ----------------
    feats_in = nc.dram_tensor("feats_in", [F, R], bf16, kind="ExternalInput").ap()
    wdr = {}
    for l in range(NLAYER):
        wdr[f"wup{l}"] = nc.dram_tensor(f"wup{l}", [1024, 1536], bf16, kind="ExternalInput").ap()
        wdr[f"wpf{l}"] = nc.dram_tensor(f"wpf{l}", [512, 256], bf16, kind="ExternalInput").ap()
        wdr[f"wxb{l}"] = nc.dram_tensor(f"wxb{l}", [512, 1280], bf16, kind="ExternalInput").ap()
        wdr[f"whb{l}"] = nc.dram_tensor(f"whb{l}", [256, 1280], bf16, kind="ExternalInput").ap()
        wdr[f"wpb{l}"] = nc.dram_tensor(f"wpb{l}", [512, 256], bf16, kind="ExternalInput").ap()
    biases_d = nc.dram_tensor("biases", [128, 52], f32, kind="ExternalInput").ap()
    coreconst_d = nc.dram_tensor("coreconst", [128, 8], f32, kind="ExternalInput").ap()
    out_d = nc.dram_tensor("out", [F, R], f32, kind="ExternalOutput").ap()

    with tile.TileContext(nc) as tc, ExitStack() as ctx:
        pool_xa = ctx.enter_context(tc.tile_pool(name="xa", bufs=1))
        pool_xb = ctx.enter_context(tc.tile_pool(name="xb", bufs=1))
        pool_c = ctx.enter_context(tc.tile_pool(name="cst", bufs=1))
        pool_w = ctx.enter_context(tc.tile_pool(name="wts", bufs=1))
        pool_misc = ctx.enter_context(tc.tile_pool(name="misc", bufs=1))
        pool_g = ctx.enter_context(tc.tile_pool(name="gat", bufs=2))
        pool_px = ctx.enter_context(tc.tile_pool(name="pxp", bufs=2))
        pool_tmp = ctx.enter_context(tc.tile_pool(name="tmp", bufs=2))
        pool_st = ctx.enter_context(tc.tile_pool(name="stg", bufs=3))
        pool_ps = ctx.enter_context(tc.tile_pool(name="psg", bufs=4, space="PSUM"))
        pool_psx = ctx.enter_context(tc.tile_pool(name="psx", bufs=2, space="PSUM"))

        # ---------------- persistent SBUF ----------------
        featsA = [pool_xa.tile([128, R], bf16, tag=f"xa{t}", name=f"fa{t}") for t in range(4)]
        featsB = [pool_xb.tile([128, R], bf16, tag=f"xb{t}", name=f"fb{t}") for t in range(4)]
        cu = [pool_c.tile([128, R], f32, tag=f"cu{t}", name=f"cu{t}") for t in range(2)]
        cd = [pool_c.tile([128, R], f32, tag=f"cd{t}", name=f"cd{t}") for t in range(2)]
        biases = pool_misc.tile([128, 52], f32, name="biases_sb")
        coreconst = pool_misc.tile([128, 8], f32, name="coreconst_sb")

        for t in range(4):
            nc.sync.dma_start(featsA[t][:], feats_in[ts(t, 128), :])
        nc.sync.dma_start(biases[:], biases_d[:])
        nc.sync.dma_start(coreconst[:], coreconst_d[:])
        maskv = coreconst[:, 0:1]
        sel4 = coreconst[:, 1:5]

        # ---------------- level-step emitters ----------------
        def up_level(wup, wpf, bcol, src, hbuf, cbuf, off, m, children,
                     extra_fix=None, store_out=False, root32=None):
            """One up-sweep level: gates for columns [off, off+m).

            children: None (leaf) or (chh[2] bf16 tiles, chc[2] f32 tiles, choff)
              with child columns [choff, choff+2m) (left=even offset).
            extra_fix: (hfe_m[2] bf16, ce_m[2] f32) single-column fix at col off.
            root32: optional [2] f32 [128,1] tiles to also receive hf (fp32).
            """
            for n0 in range(0, m, 512):
                ncv = min(512, m - n0)
                noff = off + n0
                gt = []
                for i in range(12):
                    ps = pool_ps.tile([128, ncv], f32, tag="gps", name="gps")
                    mms = []
                    for t in range(4):
                        mms.append((wup[t][:, ts(i, 128)],
                                    src[t][:, noff:noff + ncv]))
                    if children is not None:
                        chh, chc, choff = children
                        base = choff + 2 * n0
                        for t in range(4):
                            tile_idx, parity = t % 2, t // 2
                            s0 = base + parity
                            rhs = chh[tile_idx][:, s0: s0 + 2 * ncv - 1: 2]
                            mms.append((wup[4 + t][:, ts(i, 128)], rhs))
                    if extra_fix is not None and n0 == 0:
                        hfe_m, _ = extra_fix
                        for t in range(2):
                            mms.append((wup[4 + t][:, ts(i, 128)], hfe_m[t][:],
                                        ps[:, 0:1]))
                    for j, mm in enumerate(mms):
                        dst = mm[2] if len(mm) > 2 else ps[:]
                        nc.tensor.matmul(dst, mm[0], mm[1],
                                         start=(j == 0), stop=(j == len(mms) - 1))
                    g = pool_g.tile([128, ncv], f32, tag=f"g{i}", name=f"g{i}")
                    func = AF.Tanh if i in (8, 9) else AF.Sigmoid
                    nc.scalar.activation(g[:], ps[:], func,
                                         bias=biases[:, bcol + i:bcol + i + 1])
                    gt.append(g)
                pxt = []
                for t2 in range(2):
                    psx = pool_psx.tile([128, ncv], f32, tag="pxps", name="pxps")
                    for t in range(4):
                        nc.tensor.matmul(psx[:], wpf[t][:, ts(t2, 128)],
                                         src[t][:, noff:noff + ncv],
                                         start=(t == 0), stop=(t == 3))
                    px = pool_px.tile([128, ncv], f32, tag=f"px{t2}", name=f"px{t2}")
                    nc.scalar.activation(px[:], psx[:], AF.Identity,
                                         bias=biases[:, bcol + 12 + t2:bcol + 13 + t2])
                    pxt.append(px)
                for t2 in range(2):
                    ig, og, flg, frg, ug, rg = (gt[0 + t2], gt[2 + t2], gt[4 + t2],
                                                gt[6 + t2], gt[8 + t2], gt[10 + t2])
                    cdst = cbuf[t2][:, noff:noff + ncv]
                    nc.vector.tensor_mul(cdst, ig[:], ug[:])
                    if children is not None:
                        chh, chc, choff = children
                        base = choff + 2 * n0
                        for parity, fgate in ((0, flg), (1, frg)):
                            s0 = base + parity
                            cch = chc[t2][:, s0: s0 + 2 * ncv - 1: 2]
                            tmp = pool_tmp.tile([128, ncv], f32, tag=f"tmpa{t2}", name="tmpa")
                            nc.vector.tensor_mul(tmp[:], fgate[:], cch)
                            nc.vector.tensor_add(cdst, cdst, tmp[:])
                    if extra_fix is not None and n0 == 0:
                        _, ce_m = extra_fix
                        tmp1 = pool_tmp.tile([128, 1], f32, tag=f"tmpe{t2}", name="tmpe")
                        nc.vector.tensor_mul(tmp1[:], flg[:, 0:1], ce_m[t2][:])
                        nc.vector.tensor_add(cbuf[t2][:, noff:noff + 1],
                                             cbuf[t2][:, noff:noff + 1], tmp1[:])
                    th = pool_tmp.tile([128, ncv], f32, tag=f"th{t2}", name="th")
                    nc.scalar.activation(th[:], cdst, AF.Tanh)
                    hh = pool_tmp.tile([128, ncv], f32, tag=f"hh{t2}", name="hh")
                    nc.vector.tensor_mul(hh[:], og[:], th[:])
                    nc.vector.tensor_sub(hh[:], hh[:], pxt[t2][:])
                    nc.vector.tensor_mul(hh[:], rg[:], hh[:])
                    # hf = px + r*(h - px)
                    if store_out:
                        st = pool_st.tile([128, ncv], f32, tag=f"st{t2}", name="st")
                        nc.vector.tensor_add(st[:], pxt[t2][:], hh[:])
                        nc.sync.dma_start(out_d[ts(t2, 128), noff:noff + ncv], st[:])
                        nc.vector.tensor_copy(hbuf[t2][:, noff:noff + ncv], st[:])
                        if root32 is not None:
                            nc.vector.tensor_copy(root32[t2][:], st[:, 0:1])
                    else:
                        nc.vector.tensor_add(hbuf[t2][:, noff:noff + ncv],
                                             pxt[t2][:], hh[:])
                        if root32 is not None:
                            nc.vector.tensor_add(root32[t2][:], pxt[t2][:, 0:1],
                                                 hh[:, 0:1])

        def down_level(wxb, whb, wpb, bcol, src, hbuf, cbuf, off, m, parents,
                       store_out=False):
            """One down-sweep level: gates for columns [off, off+m).

            parents: None (root) or (ph[2] bf16, pc[2] f32, poff); parent of
            col off+j is column poff + j//2 (or poff when m == 1).
            """
            for n0 in range(0, m, 512):
                ncv = min(512, m - n0)
                noff = off + n0
                np2 = max(1, ncv // 2)
                gt = []
                for i in range(10):
                    ps = pool_ps.tile([128, ncv], f32, tag="gps", name="gpsd")
                    mms = [(wxb[t][:, ts(i, 128)], src[t][:, noff:noff + ncv])
                           for t in range(4)]
                    if parents is not None:
                        ph, pc, poff = parents
                        for t in range(2):
                            if m == 1:
                                rhs = ph[t][:, poff:poff + 1]
                            else:
                                p0 = poff + n0 // 2
                                rhs = ph[t][:, p0:p0 + np2].broadcast_to([128, np2, 2])
                            mms.append((whb[t][:, ts(i, 128)], rhs))
                    for j, mm in enumerate(mms):
                        nc.tensor.matmul(ps[:], mm[0], mm[1],
                                         start=(j == 0), stop=(j == len(mms) - 1))
                    g = pool_g.tile([128, ncv], f32, tag=f"g{i}", name=f"gd{i}")
                    func = AF.Tanh if i in (6, 7) else AF.Sigmoid
                    nc.scalar.activation(g[:], ps[:], func,
                                         bias=biases[:, bcol + i:bcol + i + 1])
                    gt.append(g)
                pxt = []
                for t2 in range(2):
                    psx = pool_psx.tile([128, ncv], f32, tag="pxps", name="pxpsd")
                    for t in range(4):
                        nc.tensor.matmul(psx[:], wpb[t][:, ts(t2, 128)],
                                         src[t][:, noff:noff + ncv],
                                         start=(t == 0), stop=(t == 3))
                    px = pool_px.tile([128, ncv], f32, tag=f"px{t2}", name=f"pxd{t2}")
                    nc.scalar.activation(px[:], psx[:], AF.Identity,
                                         bias=biases[:, bcol + 10 + t2:bcol + 11 + t2])
                    pxt.append(px)
                for t2 in range(2):
                    ig, og, fg, ug, rg = (gt[0 + t2], gt[2 + t2], gt[4 + t2],
                                          gt[6 + t2], gt[8 + t2])
                    cdst = cbuf[t2][:, noff:noff + ncv]
                    nc.vector.tensor_mul(cdst, ig[:], ug[:])
                    if parents is not None:
                        ph, pc, poff = parents
                        tmp = pool_tmp.tile([128, ncv], f32, tag=f"tmpa{t2}", name="tmpad")
                        if m == 1:
                            cpar = pc[t2][:, poff:poff + 1]
                            nc.vector.tensor_mul(tmp[:], fg[:], cpar)
                            nc.vector.tensor_add(cdst, cdst, tmp[:])
                        else:
                            p0 = poff + n0 // 2
                            cpar = pc[t2][:, p0:p0 + np2].broadcast_to([128, np2, 2])
                            fg3 = fg[:].rearrange("p (a b) -> p a b", b=2)
                            tmp3 = tmp[:].rearrange("p (a b) -> p a b", b=2)
                            nc.vector.tensor_mul(tmp3, fg3, cpar)
                            nc.vector.tensor_add(cdst, cdst, tmp[:])
                    th = pool_tmp.tile([128, ncv], f32, tag=f"th{t2}", name="thd")
                    nc.scalar.activation(th[:], cdst, AF.Tanh)
                    hh = pool_tmp.tile([128, ncv], f32, tag=f"hh{t2}", name="hhd")
                    nc.vector.tensor_mul(hh[:], og[:], th[:])
                    nc.vector.tensor_sub(hh[:], hh[:], pxt[t2][:])
                    nc.vector.tensor_mul(hh[:], rg[:], hh[:])
                    if store_out:
                        st = pool_st.tile([128, ncv], f32, tag=f"st{t2}", name="std")
                        nc.vector.tensor_add(st[:], pxt[t2][:], hh[:])
                        nc.sync.dma_start(out_d[ts(2 + t2, 128), noff:noff + ncv], st[:])
                        nc.vector.tensor_copy(hbuf[t2][:, noff:noff + ncv], st[:])
                    else:
                        nc.vector.tensor_add(hbuf[t2][:, noff:noff + ncv],
                                             pxt[t2][:], hh[:])

        # ---------------- the two layers ----------------
        h2u = h2d = None
        for l in range(NLAYER):
            wup = [pool_w.tile([128, 1536], bf16, tag=f"wup{t}", name=f"wup{t}") for t in range(8)]
            wpf = [pool_w.tile([128, 256], bf16, tag=f"wpf{t}", name=f"wpf{t}") for t in range(4)]
            wxb = [pool_w.tile([128, 1280], bf16, tag=f"wxb{t}", name=f"wxb{t}") for t in range(4)]
            whb = [pool_w.tile([128, 1280], bf16, tag=f"whb{t}", name=f"whb{t}") for t in range(2)]
            wpb = [pool_w.tile([128, 256], bf16, tag=f"wpb{t}", name=f"wpb{t}") for t in range(4)]
            for t in range(8):
                nc.sync.dma_start(wup[t][:], wdr[f"wup{l}"][ts(t, 128), :])
            for t in range(4):
                nc.sync.dma_start(wpf[t][:], wdr[f"wpf{l}"][ts(t, 128), :])
                nc.sync.dma_start(wxb[t][:], wdr[f"wxb{l}"][ts(t, 128), :])
                nc.sync.dma_start(wpb[t][:], wdr[f"wpb{l}"][ts(t, 128), :])
            for t in range(2):
                nc.sync.dma_start(whb[t][:], wdr[f"whb{l}"][ts(t, 128), :])

            if l == 0:
                src = featsA
                hu, hd = featsB[0:2], featsB[2:4]
            else:
                h2u = [pool_xa.tile([128, R], bf16, tag=f"xa{t}", name=f"h2u{t}") for t in range(2)]
                h2d = [pool_xa.tile([128, R], bf16, tag=f"xa{2 + t}", name=f"h2d{t}") for t in range(2)]
                src = featsB
                hu, hd = h2u, h2d
            so = (l == NLAYER - 1)
            bcu = l * 26
            bcd = l * 26 + 14

            # ---- interleaved UP + DOWN sweeps ----
            # The two sweeps are independent; emitting their level steps
            # interleaved keeps every engine fed while the other sweep sits
            # on its serial dependency (and hides the AllGather latency).
            import concourse.mybir as _mybir

            def stepU(d):
                up_level(wup, wpf, bcu, src, hu, cu, _sub_off(d), _sub_m(d),
                         (hu, cu, _sub_off(d + 1)), store_out=so,
                         root32=(root_hf32 if d == 3 else None))

            def stepD(d):
                down_level(wxb, whb, wpb, bcd, src, hd, cd, _sub_off(d), _sub_m(d),
                           (hd, cd, _sub_off(d - 1)), store_out=so)

            # up: extra leaf node (col 7) + its mask
            hfe32 = [pool_misc.tile([128, 1], f32, tag=f"hfe32{t}", name="hfe32") for t in range(2)]
            up_level(wup, wpf, bcu, src, hu, cu, 7, 1, None,
                     store_out=so, root32=hfe32)
            hfe_m = [pool_misc.tile([128, 1], bf16, tag=f"hfem{t}", name="hfem") for t in range(2)]
            ce_m = [pool_misc.tile([128, 1], f32, tag=f"cem{t}", name="cem") for t in range(2)]
            for t in range(2):
                nc.scalar.mul(hfe_m[t][:], hfe32[t][:], maskv)
                nc.scalar.mul(ce_m[t][:], cu[t][:, 7:8], maskv)
            root_hf32 = [pool_misc.tile([128, 1], f32, tag=f"rhf32{t}", name="roothf") for t in range(2)]

            down_level(wxb, whb, wpb, bcd, src, hd, cd, 0, 1, None, store_out=so)
            up_level(wup, wpf, bcu, src, hu, cu, _sub_off(13), 1024, None,
                     extra_fix=(hfe_m, ce_m), store_out=so)
            down_level(wxb, whb, wpb, bcd, src, hd, cd, 1, 2, (hd, cd, 0), store_out=so)
            stepU(12)
            down_level(wxb, whb, wpb, bcd, src, hd, cd, 3, 4, (hd, cd, 1), store_out=so)
            stepU(11)
            # down subtree root (col 8): parent = top level-2 col 3 + k//2 via one-hot
            hpar = [pool_misc.tile([128, 1], f32, tag=f"hpar{t}", name="hpar") for t in range(2)]
            hparb = [pool_misc.tile([128, 1], bf16, tag=f"hparb{t}", name="hparb") for t in range(2)]
            cpar = [pool_misc.tile([128, 1], f32, tag=f"cpar{t}", name="cpar") for t in range(2)]
            for t in range(2):
                tsel = pool_tmp.tile([128, 4], f32, tag=f"tsel{t}", name="tsel")
                nc.vector.tensor_copy(tsel[:], hd[t][:, 3:7])
                nc.vector.tensor_mul(tsel[:], tsel[:], sel4)
                nc.vector.reduce_sum(hpar[t][:], tsel[:], axis=_mybir.AxisListType.X)
                nc.vector.tensor_copy(hparb[t][:], hpar[t][:])
                tsel2 = pool_tmp.tile([128, 4], f32, tag=f"tsel2{t}", name="tsel2")
                nc.vector.tensor_mul(tsel2[:], cd[t][:, 3:7], sel4)
                nc.vector.reduce_sum(cpar[t][:], tsel2[:], axis=_mybir.AxisListType.X)
            down_level(wxb, whb, wpb, bcd, src, hd, cd, 8, 1, (hparb, cpar, 0),
                       store_out=so)
            stepU(10)
            stepD(4)
            stepU(9)
            stepD(5)
            stepU(8)
            stepD(6)
            stepU(7)
            stepD(7)
            stepU(6)
            stepD(8)
            stepU(5)
            stepD(9)
            stepU(4)
            stepD(10)
            stepU(3)
            stepD(11)

            # ---- AllGather of the 8 subtree-root (c, hf) states ----
            # (emitted before the two biggest down levels so its ~50us
            # latency hides under them)
            cc_in, _f1 = tc.tile([1, 512], f32, space="DRAM", name=f"cc_in{l}")
            ctx.callback(_f1)
            cc_out, _f2 = tc.tile([8, 512], f32, space="DRAM", addr_space="Shared",
                             name=f"cc_out{l}")
            ctx.callback(_f2)
            for t in range(2):
                nc.sync.dma_start(cc_in[0:1, ts(t, 128)].rearrange("o p -> p o"),
                                  cu[t][:, 8:9])
                nc.sync.dma_start(cc_in[0:1, ts(2 + t, 128)].rearrange("o p -> p o"),
                                  root_hf32[t][:])
            nc.gpsimd.collective_compute(
                "AllGather", _mybir.AluOpType.bypass,
                replica_groups=[list(range(NCORE))],
                ins=[cc_in[:]], outs=[cc_out[:]])
            rc = [pool_misc.tile([128, 8], f32, tag=f"rc{t}", name="rc") for t in range(2)]
            rhf = [pool_misc.tile([128, 8], f32, tag=f"rhf{t}", name="rhf") for t in range(2)]
            rhb = [pool_misc.tile([128, 8], bf16, tag=f"rhb{t}", name="rhb") for t in range(2)]
            for t in range(2):
                nc.sync.dma_start(rc[t][:], cc_out[:, ts(t, 128)].rearrange("j p -> p j"))
                nc.sync.dma_start(rhf[t][:], cc_out[:, ts(2 + t, 128)].rearrange("j p -> p j"))
                nc.vector.tensor_copy(rhb[t][:], rhf[t][:])

            stepD(12)
            stepD(13)
            up_level(wup, wpf, bcu, src, hu, cu, 3, 4, (rhb, rc, 0), store_out=so)
            up_level(wup, wpf, bcu, src, hu, cu, 1, 2, (hu, cu, 3), store_out=so)
            up_level(wup, wpf, bcu, src, hu, cu, 0, 1, (hu, cu, 1), store_out=so)
            # level-14 node (col 7) down: parent is subtree level-13 col 0 (col 1031)
            down_level(wxb, whb, wpb, bcd, src, hd, cd, 7, 1,
                       (hd, cd, _sub_off(13)), store_out=so)

    nc.compile()
    return nc


def _prep_inputs(features, Wxf, bxf, Wlf, blf, Wrf, brf, Wpf, bpf,
                 Wxb, bxb, Whb, bhb, Wpb, bpb):
    """Host-side sharding: build the per-core input maps."""
    features = np.asarray(features, np.float32)
    in_maps = []
    shared = {}
    for l in range(NLAYER):
        wcomb = np.concatenate([np.asarray(Wlf[l]), np.asarray(Wrf[l])], axis=1)
        wup = np.concatenate([np.asarray(Wxf[l]).T, wcomb.T], axis=0)
        shared[f"wup{l}"] = np.ascontiguousarray(wup).astype(BF16)
        shared[f"wpf{l}"] = np.ascontiguousarray(np.asarray(Wpf[l]).T).astype(BF16)
        shared[f"wxb{l}"] = np.ascontiguousarray(np.asarray(Wxb[l]).T).astype(BF16)
        shared[f"whb{l}"] = np.ascontiguousarray(np.asarray(Whb[l]).T).astype(BF16)
        shared[f"wpb{l}"] = np.ascontiguousarray(np.asarray(Wpb[l]).T).astype(BF16)
    biases = np.zeros((128, 52), np.float32)
    for l in range(NLAYER):
        bup = (np.asarray(bxf[l]) + np.asarray(blf[l]) + np.asarray(brf[l])).astype(np.float32)
        bdn = (np.asarray(bxb[l]) + np.asarray(bhb[l])).astype(np.float32)
        biases[:, l * 26 + 0:l * 26 + 12] = bup.reshape(12, 128).T
        biases[:, l * 26 + 12:l * 26 + 14] = np.asarray(bpf[l], np.float32).reshape(2, 128).T
        biases[:, l * 26 + 14:l * 26 + 24] = bdn.reshape(10, 128).T
        biases[:, l * 26 + 24:l * 26 + 26] = np.asarray(bpb[l], np.float32).reshape(2, 128).T
    for k in range(NCORE):
        cols = _col_nodes_for_core(k)
        fk = np.ascontiguousarray(features[cols, :].T).astype(BF16)
        cc = np.zeros((128, 8), np.float32)
        cc[:, 0] = 1.0 if k == 0 else 0.0
        cc[:, 1 + (k // 2)] = 1.0
        m = dict(shared)
        m["feats_in"] = fk
        m["biases"] = biases
        m["coreconst"] = cc
        in_maps.append(m)
    return in_maps


def _assemble_output(results):
    """Gather per-core [F, R] outputs back to the full [N, F] array."""
    out = np.empty((N, F), np.float32)
    for k in range(NCORE):
        cols = _col_nodes_for_core(k)
        ok = results[k]["out"]  # [F, R]
        if k == 0:
            out[cols, :] = ok.T
        else:
            out[cols[8:], :] = ok.T[8:, :]
    return out


def kernel(features, left, right, parent, Wxf, bxf, Wlf, blf, Wrf, brf,
           Wpf, bpf, Wxb, bxb, Whb, bhb, Wpb, bpb):
    from concourse import bass_utils
    nc = build_nc()
    in_maps = _prep_inputs(features, Wxf, bxf, Wlf, blf, Wrf, brf, Wpf, bpf,
                           Wxb, bxb, Whb, bhb, Wpb, bpb)
    res = bass_utils.run_bass_kernel_spmd(nc, in_maps, core_ids=list(range(NCORE)))
    return _assemble_output(res.results)
